# revision 40
# baseline (speedup 1.0000x reference)
"""Trainium2 Bass kernel for nn_MixvMFGrad (mixture-of-vMF log-density gradient).

Math (per row s of the batch, d=512, K=64 components):
    dots  = s @ mus^T                       [K]
    t_k   = delta_k + kappa_k * dots_k      (delta = coef - max coef, folded on host)
    e     = exp(t)                          (|t| <= ~35 by construction)
    g     = e @ mus                         [d]
    q     = g . s  = sum_k e_k * dots_k
    n2    = |g|^2  = |L^T e|^2,  G = mus mus^T = L L^T (host Cholesky)
    out   = (g - q s) / sqrt(n2)

Memory-regime problem: per core 25088 rows x 512 in + out.  Everything runs
in bf16 (rel err ~7e-3 vs the 2e-2 gate): HBM traffic halves vs fp32 and the
PE runs matmuls at 1 col/cycle.

The whole pipeline works on TRANSPOSED supertiles: the host sends each
512-row supertile as s^T [512d, 512r] (and un-transposes the output), so the
device never transposes or copies s -- the PE reads s^T chunks directly as
the moving operand of the dots matmuls.  (A previous row-major version spent
~45% of PE time on 16 transposes/supertile plus their LDWEIGHTS and
PSUM->SBUF evacuations.)  Per supertile:
    A    = dots2^T [K, rows]      4 chunk matmuls, kappa folded into wk
    e    = exp(A + delta)         ACT, bf16
    W    = L^T e                  PE;  W2 = Square(W)  ACT -> up[64:128]
    u    = e * A                  DVE            -> up[0:64]
    qn_j = up-chunk^T @ [-1/kappa | 0; 0 | 1] = (-q_j | n2_j)   [128, 2] PE
    r    = rsqrt(n2)  DVE bit-trick + 1 Newton;  rq = r*(-q)    [128,4] bf16
    r,rq -> [2, 512] row vectors via 4 tiny PE pair-transposes, then
            materialized as real broadcast tiles rbc [64,512] / rqbc [128,512]
            by two selection matmuls (partition-broadcast APs are illegal)
    e_r  = e * rbc                DVE  (fold the 1/|g| scale into e)
    gT_c = s^T_c * rqbc           DVE seeds the tangent-projection term into
           += mus_c^T @ e_r       PSUM; PE accumulates the r-scaled g on top
    out  = ACT copy gT -> bf16    the only output-side pointwise pass
"""

import os
from contextlib import ExitStack

import numpy as np
import ml_dtypes

import concourse.bass as bass
import concourse.tile as tile
from concourse import bacc
from concourse import mybir
from concourse.bass_utils import run_bass_kernel_spmd

N_CORES = 8
BS = 200000
D = 512
K = 64
ROWS_PER_CORE = BS // N_CORES  # 25000
ST_ROWS = 512                  # rows per supertile
N_ST = 49
PAD_ROWS = N_ST * ST_ROWS      # 25088
F32 = mybir.dt.float32
BF16 = mybir.dt.bfloat16
NPBF16 = ml_dtypes.bfloat16

LAST_RESULT = None  # test.py reads exec_time_ns off this


def build_nc(rows=PAD_ROWS):
    assert rows % ST_ROWS == 0
    n_st = rows // ST_ROWS
    nc = bacc.Bacc("TRN2", target_bir_lowering=False)

    sT_d = nc.dram_tensor("sT", [rows, D], BF16, kind="ExternalInput")
    out_d = nc.dram_tensor("outT", [rows, D], BF16, kind="ExternalOutput")
    wk_d = nc.dram_tensor("wk", [128, 4, K], BF16, kind="ExternalInput")
    musr_d = nc.dram_tensor("musr", [K, D], BF16, kind="ExternalInput")
    lmat_d = nc.dram_tensor("lmat", [K, K], BF16, kind="ExternalInput")
    delta_d = nc.dram_tensor("delta", [K, 1], F32, kind="ExternalInput")
    iv2_d = nc.dram_tensor("iv2", [128, 2], BF16, kind="ExternalInput")
    sel_d = nc.dram_tensor("sel", [2, 256], BF16, kind="ExternalInput")
    ident_d = nc.dram_tensor("ident", [128, 128], BF16, kind="ExternalInput")

    AF = mybir.ActivationFunctionType
    OP = mybir.AluOpType

    # transposed supertiles: DRAM row t*512 + c*128 + p holds s[t*512 + r, c*128 + p]
    s_v = sT_d[:].rearrange("(t c p) r -> t p c r", p=128, c=4)
    o_v = out_d[:].rearrange("(t c p) r -> t p c r", p=128, c=4)

    with tile.TileContext(nc) as tc, ExitStack() as ctx:
        consts = ctx.enter_context(tc.tile_pool(name="consts", bufs=1))
        in_pool = ctx.enter_context(tc.tile_pool(name="in_pool", bufs=4))
        out_pool = ctx.enter_context(tc.tile_pool(name="out_pool", bufs=4))
        small = ctx.enter_context(tc.tile_pool(name="small", bufs=3))
        ps_A = ctx.enter_context(tc.tile_pool(name="ps_A", bufs=1, space="PSUM"))
        ps_W = ctx.enter_context(tc.tile_pool(name="ps_W", bufs=1, space="PSUM"))
        ps_q = ctx.enter_context(tc.tile_pool(name="ps_q", bufs=1, space="PSUM"))
        ps_rb = ctx.enter_context(tc.tile_pool(name="ps_rb", bufs=1, space="PSUM"))
        ps_rq = ctx.enter_context(tc.tile_pool(name="ps_rq", bufs=1, space="PSUM"))
        ps_g = ctx.enter_context(tc.tile_pool(name="ps_g", bufs=3, space="PSUM"))

        wk_sb = consts.tile([128, 4, K], BF16)
        nc.sync.dma_start(out=wk_sb, in_=wk_d[:])
        musr_sb = consts.tile([K, D], BF16)
        nc.sync.dma_start(out=musr_sb, in_=musr_d[:])
        lmat_sb = consts.tile([K, K], BF16)
        nc.sync.dma_start(out=lmat_sb, in_=lmat_d[:])
        delta_sb = consts.tile([K, 1], F32)
        nc.sync.dma_start(out=delta_sb, in_=delta_d[:])
        iv2_sb = consts.tile([128, 2], BF16)
        nc.sync.dma_start(out=iv2_sb, in_=iv2_d[:])
        sel_sb = consts.tile([2, 256], BF16)
        nc.sync.dma_start(out=sel_sb, in_=sel_d[:])
        ident_sb = consts.tile([128, 128], BF16)
        nc.sync.dma_start(out=ident_sb, in_=ident_d[:])

        for st in range(n_st):
            sT_t = in_pool.tile([128, 4, D], BF16, tag="sT")
            nc.sync.dma_start(out=sT_t, in_=s_v[st])
            oT_t = out_pool.tile([128, 4, D], BF16, tag="oT")

            # A = dots2^T [K, 512] accumulated over 4 d-chunks (s^T direct)
            A = ps_A.tile([K, D], F32, tag="A")
            for c in range(4):
                nc.tensor.matmul(
                    A, wk_sb[:, c, :], sT_t[:, c, :],
                    start=(c == 0), stop=(c == 3),
                )

            e_t = small.tile([K, D], BF16, tag="e")
            nc.scalar.activation(e_t, A, AF.Exp, bias=delta_sb)

            # W = L^T e  (|W|^2 = e^T G e = |g|^2)
            W = ps_W.tile([K, D], F32, tag="W")
            nc.tensor.matmul(W, lmat_sb, e_t, start=True, stop=True)

            # stacked [u; W2]: one subtile matmul against
            # iv2 = [-1/kappa | 0; 0 | 1] gives col 2j = -q_j, col 2j+1 = n2_j
            up_t = small.tile([128, D], BF16, tag="up")
            nc.vector.tensor_mul(up_t[0:64, :], e_t, A)            # e * dots2
            nc.scalar.activation(up_t[64:128, :], W, AF.Square)    # (L^T e)^2

            # one PSUM bank shared by qn [128, :8] and the r/rq row pair
            # rT2 [2, 512] bf16 (stored bitcast at f32 cols 8:264)
            qrt = ps_q.tile([128, 512], F32, tag="qrt")
            for j in range(4):
                nc.tensor.matmul(
                    qrt[:, 2 * j:2 * j + 2],
                    up_t[:, 128 * j:128 * (j + 1)], iv2_sb,
                    start=True, stop=True)
            qr_sb = small.tile([128, 8], F32, tag="qr")
            nc.scalar.copy(qr_sb, qrt[:, 0:8])
            qr_v = qr_sb.rearrange("p (j c) -> p j c", c=2)

            # r = rsqrt(n2) on DVE: bit-trick seed + 1 Newton step
            # ([128,4] tiles, all ops tiny); r/rq written as bf16 pairs
            nr = small.tile([128, 12], F32, tag="nr")
            rr_bf = small.tile([128, 4, 2], BF16, tag="rr")
            x = qr_v[:, :, 1]
            xi = x.bitcast(mybir.dt.int32)
            y0i = nr[:, 0:4].bitcast(mybir.dt.int32)
            nc.vector.tensor_scalar(
                out=nr[:, 8:12].bitcast(mybir.dt.int32), in0=xi,
                scalar1=1, scalar2=None, op0=OP.arith_shift_right)
            nc.vector.tensor_scalar(
                out=y0i, in0=nr[:, 8:12].bitcast(mybir.dt.int32),
                scalar1=-1, scalar2=0x5F3759DF, op0=OP.mult, op1=OP.add)
            y = nr[:, 0:4]
            h1 = nr[:, 4:8]
            nc.vector.tensor_mul(h1, x, y)        # x*y
            nc.vector.tensor_mul(h1, h1, y)       # x*y^2
            nc.vector.tensor_scalar(
                out=h1, in0=h1, scalar1=-0.5, scalar2=1.5,
                op0=OP.mult, op1=OP.add)          # 1.5 - 0.5*x*y^2
            nc.vector.tensor_mul(rr_bf[:, :, 0], h1, y)                 # r
            nc.vector.tensor_mul(rr_bf[:, :, 1], rr_bf[:, :, 0], qr_v[:, :, 0])  # r*(-q)

            # [r; rq] -> [2, 512] row pair via 4 tiny PE pair-transposes
            rT2 = qrt[0:2, 8:264].bitcast(BF16)    # [2, 512] bf16 view
            for j in range(4):
                nc.tensor.transpose(
                    rT2[:, 128 * j:128 * (j + 1)], rr_bf[:, j, :], ident_sb)
            rT2_sb = small.tile([2, D], BF16, tag="rt2")
            nc.scalar.copy(rT2_sb.bitcast(F32), rT2.bitcast(F32))

            # materialize broadcast tiles (partition-broadcast APs are
            # illegal): rbc [64, 512] = r row, rqbc [128, 512] = rq row
            rbc = ps_rb.tile([K, D], F32, tag="rb")
            nc.tensor.matmul(rbc, sel_sb[:, 0:K], rT2_sb, start=True, stop=True)
            rqbc = ps_rq.tile([128, D], F32, tag="rq")
            nc.tensor.matmul(rqbc, sel_sb[:, 128:256], rT2_sb,
                             start=True, stop=True)

            # e_r = e * r_row  (fold the normalization into the g matmul)
            er_t = small.tile([K, D], BF16, tag="er")
            nc.vector.tensor_mul(er_t, e_t, rbc)

            for c in range(4):
                gT = ps_g.tile([128, D], F32, tag="g")
                # seed the tangent-projection term, PE accumulates r*g on top
                nc.vector.tensor_mul(gT, sT_t[:, c, :], rqbc)
                nc.tensor.matmul(
                    gT, musr_sb[:, 128 * c:128 * (c + 1)], er_t,
                    start=False, stop=True, skip_group_check=True)
                nc.scalar.copy(oT_t[:, c, :], gT)

            nc.sync.dma_start(out=o_v[st], in_=oT_t)

    nc.finalize()
    return nc


def host_prep(alphas, mus, kappas):
    """Host-side fp64 precompute of the tiny per-component constants."""
    a = np.asarray(alphas, np.float64)
    m = np.asarray(mus, np.float64)
    k = np.asarray(kappas, np.float64)
    d = m.shape[1]
    nu = 0.5 * d - 1.0
    z = k / nu
    sq = np.sqrt(1.0 + z * z)
    eta = sq + np.log(z) - np.log1p(sq)
    t = 1.0 / sq
    u1 = (3.0 * t - 5.0 * t ** 3) / 24.0
    u2 = (81.0 * t ** 2 - 462.0 * t ** 4 + 385.0 * t ** 6) / 1152.0
    log_iv = (nu * eta - 0.5 * np.log(2.0 * np.pi * nu)
              - 0.25 * np.log1p(z * z) + np.log1p(u1 / nu + u2 / (nu * nu)))
    logC = d * (-0.5 * np.log(2.0 * np.pi)) + nu * np.log(k) - log_iv
    coef = np.log(a) + np.log(k) + logC
    delta = (coef - coef.max()).astype(np.float32).reshape(K, 1)

    musk = (k[:, None] * m)                    # kappa_k * mus_k
    # wk[p, c, j] = musk[j, 128c + p]
    wk = np.ascontiguousarray(
        musk.reshape(K, 4, 128).transpose(2, 1, 0)).astype(NPBF16)
    musr = m.astype(NPBF16)
    gram = m @ m.T
    # jittered Cholesky: G = L L^T, |g|^2 = |L^T e|^2
    lmat = np.linalg.cholesky(gram + 1e-9 * np.eye(K)).astype(NPBF16)
    iv2 = np.zeros((128, 2), np.float64)       # [-1/kappa | 0; 0 | 1] stacked
    iv2[0:64, 0] = -1.0 / k
    iv2[64:128, 1] = 1.0
    iv2 = iv2.astype(NPBF16)
    # sel[:, 0:128] selects the r row (rbc); sel[:, 128:256] the rq row (rqbc)
    sel = np.zeros((2, 256), np.float64)
    sel[0, 0:128] = 1.0    # rbc[m, :] = rT2[0, :] = r
    sel[1, 128:256] = 1.0  # rqbc[m, :] = rT2[1, :] = r*(-q)
    sel_r = sel.astype(NPBF16)
    ident = np.eye(128).astype(NPBF16)
    return dict(wk=wk, musr=musr, lmat=lmat, delta=delta, iv2=iv2,
                sel=sel_r, ident=ident)


_NC_CACHE = {}


def kernel(s, alphas, mus, kappas):
    global LAST_RESULT
    s = np.asarray(s, np.float32)
    consts = host_prep(alphas, mus, kappas)

    rows = PAD_ROWS
    if rows not in _NC_CACHE:
        _NC_CACHE[rows] = build_nc(rows)
    nc = _NC_CACHE[rows]

    in_maps = []
    for c in range(N_CORES):
        shard = s[c * ROWS_PER_CORE:(c + 1) * ROWS_PER_CORE]
        pad = rows - shard.shape[0]
        if pad:
            shard = np.concatenate([shard, shard[:pad]], axis=0)
        # per-supertile transpose: DRAM holds s^T blocks [512d, 512r]
        sT = np.ascontiguousarray(
            shard.reshape(N_ST, ST_ROWS, D).transpose(0, 2, 1)
        ).astype(NPBF16).reshape(rows, D)
        in_maps.append({"sT": sT, **consts})

    res = run_bass_kernel_spmd(
        nc, in_maps, list(range(N_CORES)),
        trace=bool(os.environ.get("MIXVMF_TRACE")),
    )
    LAST_RESULT = res
    outs = []
    for c in range(N_CORES):
        oT = np.asarray(res.results[c]["outT"]).reshape(N_ST, D, ST_ROWS)
        o = oT.transpose(0, 2, 1).reshape(rows, D)[:ROWS_PER_CORE]
        outs.append(o.astype(np.float32))
    return np.concatenate(outs, axis=0)


# revision 47
# speedup vs baseline: 1.0358x; 1.0358x over previous
"""Trainium2 Bass kernel for nn_MixvMFGrad (mixture-of-vMF log-density gradient).

Math (per row s of the batch, d=512, K=64 components):
    dots  = s @ mus^T                       [K]
    t_k   = delta_k + kappa_k * dots_k      (delta = coef - max coef, folded on host)
    e     = exp(t)                          (|t| <= ~35 by construction)
    g     = e @ mus                         [d]
    q     = g . s  = sum_k e_k * dots_k
    n2    = |g|^2  = |L^T e|^2,  G = mus mus^T = L L^T (host Cholesky)
    out   = (g - q s) / sqrt(n2)

Memory-regime problem: per core 25088 rows x 512 in + out.  Everything runs
in bf16 (rel err ~7e-3 vs the 2e-2 gate): HBM traffic halves vs fp32 and the
PE runs matmuls at 1 col/cycle.

The whole pipeline works on TRANSPOSED supertiles: the host sends each
512-row supertile as s^T [512d, 512r] (and un-transposes the output), so the
device never transposes or copies s -- the PE reads s^T chunks directly as
the moving operand of the dots matmuls.  (A previous row-major version spent
~45% of PE time on 16 transposes/supertile plus their LDWEIGHTS and
PSUM->SBUF evacuations.)  Per supertile:
    A    = dots2^T [K, rows]      4 chunk matmuls, kappa folded into wk
    e    = exp(A + delta)         ACT, bf16
    W    = L^T e                  PE;  W2 = Square(W)  ACT -> up[64:128]
    u    = e * A                  DVE            -> up[0:64]
    qn_j = up-chunk^T @ [-1/kappa | 0; 0 | 1] = (-q_j | n2_j)   [128, 2] PE
    r    = rsqrt(n2)  DVE bit-trick + 1 Newton;  rq = r*(-q)    [128,4] bf16
    r,rq -> [2, 512] row vectors via 4 tiny PE pair-transposes, then
            materialized as real broadcast tiles rbc [64,512] / rqbc [128,512]
            by two selection matmuls (partition-broadcast APs are illegal)
    e_r  = e * rbc                DVE  (fold the 1/|g| scale into e)
    gT_c = s^T_c * rqbc           DVE seeds the tangent-projection term into
           += mus_c^T @ e_r       PSUM; PE accumulates the r-scaled g on top
    out  = ACT copy gT -> bf16    the only output-side pointwise pass
"""

import os
from contextlib import ExitStack

import numpy as np
import ml_dtypes

import concourse.bass as bass
import concourse.tile as tile
from concourse import bacc
from concourse import mybir
from concourse.bass_utils import run_bass_kernel_spmd

N_CORES = 8
BS = 200000
D = 512
K = 64
ROWS_PER_CORE = BS // N_CORES  # 25000
ST_ROWS = 512                  # rows per supertile
N_ST = 49
PAD_ROWS = N_ST * ST_ROWS      # 25088
F32 = mybir.dt.float32
BF16 = mybir.dt.bfloat16
NPBF16 = ml_dtypes.bfloat16

LAST_RESULT = None  # test.py reads exec_time_ns off this


def build_nc(rows=PAD_ROWS):
    assert rows % ST_ROWS == 0
    n_st = rows // ST_ROWS
    nc = bacc.Bacc("TRN2", target_bir_lowering=False)

    sT_d = nc.dram_tensor("sT", [rows, D], BF16, kind="ExternalInput")
    out_d = nc.dram_tensor("outT", [rows, D], BF16, kind="ExternalOutput")
    wk_d = nc.dram_tensor("wk", [128, 4, K], BF16, kind="ExternalInput")
    musr_d = nc.dram_tensor("musr", [K, D], BF16, kind="ExternalInput")
    lmat_d = nc.dram_tensor("lmat", [K, K], BF16, kind="ExternalInput")
    delta_d = nc.dram_tensor("delta", [K, 1], F32, kind="ExternalInput")
    iv2_d = nc.dram_tensor("iv2", [128, 2], BF16, kind="ExternalInput")
    sel_d = nc.dram_tensor("sel", [2, 256], BF16, kind="ExternalInput")
    ident_d = nc.dram_tensor("ident", [128, 128], BF16, kind="ExternalInput")

    AF = mybir.ActivationFunctionType
    OP = mybir.AluOpType

    # transposed supertiles: DRAM row t*512 + c*128 + p holds s[t*512 + r, c*128 + p]
    s_v = sT_d[:].rearrange("(t c p) r -> t p c r", p=128, c=4)
    o_v = out_d[:].rearrange("(t c p) r -> t p c r", p=128, c=4)

    with tile.TileContext(nc) as tc, ExitStack() as ctx:
        consts = ctx.enter_context(tc.tile_pool(name="consts", bufs=1))
        in_pool = ctx.enter_context(tc.tile_pool(name="in_pool", bufs=6))
        out_pool = ctx.enter_context(tc.tile_pool(name="out_pool", bufs=6))
        small = ctx.enter_context(tc.tile_pool(name="small", bufs=4))
        ps_A = ctx.enter_context(tc.tile_pool(name="ps_A", bufs=2, space="PSUM"))
        ps_W = ctx.enter_context(tc.tile_pool(name="ps_W", bufs=1, space="PSUM"))
        ps_q = ctx.enter_context(tc.tile_pool(name="ps_q", bufs=1, space="PSUM"))
        ps_rb = ctx.enter_context(tc.tile_pool(name="ps_rb", bufs=1, space="PSUM"))
        ps_rq = ctx.enter_context(tc.tile_pool(name="ps_rq", bufs=1, space="PSUM"))
        ps_g = ctx.enter_context(tc.tile_pool(name="ps_g", bufs=2, space="PSUM"))

        wk_sb = consts.tile([128, 4, K], BF16)
        nc.sync.dma_start(out=wk_sb, in_=wk_d[:])
        musr_sb = consts.tile([K, D], BF16)
        nc.sync.dma_start(out=musr_sb, in_=musr_d[:])
        lmat_sb = consts.tile([K, K], BF16)
        nc.sync.dma_start(out=lmat_sb, in_=lmat_d[:])
        delta_sb = consts.tile([K, 1], F32)
        nc.sync.dma_start(out=delta_sb, in_=delta_d[:])
        iv2_sb = consts.tile([128, 2], BF16)
        nc.sync.dma_start(out=iv2_sb, in_=iv2_d[:])
        sel_sb = consts.tile([2, 256], BF16)
        nc.sync.dma_start(out=sel_sb, in_=sel_d[:])
        ident_sb = consts.tile([128, 128], BF16)
        nc.sync.dma_start(out=ident_sb, in_=ident_d[:])

        for st in range(n_st):
            sT_t = in_pool.tile([128, 4, D], BF16, tag="sT")
            nc.sync.dma_start(out=sT_t, in_=s_v[st])
            oT_t = out_pool.tile([128, 4, D], BF16, tag="oT")

            # A = dots2^T [K, 512] accumulated over 4 d-chunks (s^T direct)
            A = ps_A.tile([K, D], F32, tag="A")
            for c in range(4):
                nc.tensor.matmul(
                    A, wk_sb[:, c, :], sT_t[:, c, :],
                    start=(c == 0), stop=(c == 3),
                )

            e_t = small.tile([K, D], BF16, tag="e")
            nc.scalar.activation(e_t, A, AF.Exp, bias=delta_sb)

            # W = L^T e  (|W|^2 = e^T G e = |g|^2)
            W = ps_W.tile([K, D], F32, tag="W")
            nc.tensor.matmul(W, lmat_sb, e_t, start=True, stop=True)

            # stacked [u; W2]: one subtile matmul against
            # iv2 = [-1/kappa | 0; 0 | 1] gives col 2j = -q_j, col 2j+1 = n2_j
            up_t = small.tile([128, D], BF16, tag="up")
            nc.vector.tensor_mul(up_t[0:64, :], e_t, A)            # e * dots2
            nc.scalar.activation(up_t[64:128, :], W, AF.Square)    # (L^T e)^2

            # one PSUM bank shared by qn [128, :8] and the r/rq row pair
            # rT2 [2, 512] bf16 (stored bitcast at f32 cols 8:264)
            qrt = ps_q.tile([128, 512], F32, tag="qrt")
            for j in range(4):
                nc.tensor.matmul(
                    qrt[:, 2 * j:2 * j + 2],
                    up_t[:, 128 * j:128 * (j + 1)], iv2_sb,
                    start=True, stop=True)
            qr_sb = small.tile([128, 8], F32, tag="qr")
            nc.scalar.copy(qr_sb, qrt[:, 0:8])
            qr_v = qr_sb.rearrange("p (j c) -> p j c", c=2)

            # r = rsqrt(n2) on DVE: bit-trick seed + 1 Newton step
            # ([128,4] tiles, all ops tiny); r/rq written as bf16 pairs
            nr = small.tile([128, 12], F32, tag="nr")
            rr_bf = small.tile([128, 4, 2], BF16, tag="rr")
            x = qr_v[:, :, 1]
            xi = x.bitcast(mybir.dt.int32)
            y0i = nr[:, 0:4].bitcast(mybir.dt.int32)
            nc.vector.tensor_scalar(
                out=nr[:, 8:12].bitcast(mybir.dt.int32), in0=xi,
                scalar1=1, scalar2=None, op0=OP.arith_shift_right)
            nc.vector.tensor_scalar(
                out=y0i, in0=nr[:, 8:12].bitcast(mybir.dt.int32),
                scalar1=-1, scalar2=0x5F3759DF, op0=OP.mult, op1=OP.add)
            y = nr[:, 0:4]
            h1 = nr[:, 4:8]
            nc.vector.tensor_mul(h1, x, y)        # x*y
            nc.vector.tensor_mul(h1, h1, y)       # x*y^2
            nc.vector.tensor_scalar(
                out=h1, in0=h1, scalar1=-0.5, scalar2=1.5,
                op0=OP.mult, op1=OP.add)          # 1.5 - 0.5*x*y^2
            nc.vector.tensor_mul(rr_bf[:, :, 0], h1, y)                 # r
            nc.vector.tensor_mul(rr_bf[:, :, 1], rr_bf[:, :, 0], qr_v[:, :, 0])  # r*(-q)

            # [r; rq] -> [2, 512] row pair via 4 tiny PE pair-transposes
            rT2 = qrt[0:2, 8:264].bitcast(BF16)    # [2, 512] bf16 view
            for j in range(4):
                nc.tensor.transpose(
                    rT2[:, 128 * j:128 * (j + 1)], rr_bf[:, j, :], ident_sb)
            rT2_sb = small.tile([2, D], BF16, tag="rt2")
            nc.scalar.copy(rT2_sb.bitcast(F32), rT2.bitcast(F32))

            # materialize broadcast tiles (partition-broadcast APs are
            # illegal): rbc [64, 512] = r row, rqbc [128, 512] = rq row
            rbc = ps_rb.tile([K, D], F32, tag="rb")
            nc.tensor.matmul(rbc, sel_sb[:, 0:K], rT2_sb, start=True, stop=True)
            rqbc = ps_rq.tile([128, D], F32, tag="rq")
            nc.tensor.matmul(rqbc, sel_sb[:, 128:256], rT2_sb,
                             start=True, stop=True)

            # e_r = e * r_row  (fold the normalization into the g matmul)
            er_t = small.tile([K, D], BF16, tag="er")
            nc.vector.tensor_mul(er_t, e_t, rbc)

            for c in range(4):
                gT = ps_g.tile([128, D], F32, tag="g")
                # seed the tangent-projection term, PE accumulates r*g on top
                nc.vector.tensor_mul(gT, sT_t[:, c, :], rqbc)
                nc.tensor.matmul(
                    gT, musr_sb[:, 128 * c:128 * (c + 1)], er_t,
                    start=False, stop=True, skip_group_check=True)
                nc.scalar.copy(oT_t[:, c, :], gT)

            nc.sync.dma_start(out=o_v[st], in_=oT_t)

    nc.finalize()
    return nc


def host_prep(alphas, mus, kappas):
    """Host-side fp64 precompute of the tiny per-component constants."""
    a = np.asarray(alphas, np.float64)
    m = np.asarray(mus, np.float64)
    k = np.asarray(kappas, np.float64)
    d = m.shape[1]
    nu = 0.5 * d - 1.0
    z = k / nu
    sq = np.sqrt(1.0 + z * z)
    eta = sq + np.log(z) - np.log1p(sq)
    t = 1.0 / sq
    u1 = (3.0 * t - 5.0 * t ** 3) / 24.0
    u2 = (81.0 * t ** 2 - 462.0 * t ** 4 + 385.0 * t ** 6) / 1152.0
    log_iv = (nu * eta - 0.5 * np.log(2.0 * np.pi * nu)
              - 0.25 * np.log1p(z * z) + np.log1p(u1 / nu + u2 / (nu * nu)))
    logC = d * (-0.5 * np.log(2.0 * np.pi)) + nu * np.log(k) - log_iv
    coef = np.log(a) + np.log(k) + logC
    delta = (coef - coef.max()).astype(np.float32).reshape(K, 1)

    musk = (k[:, None] * m)                    # kappa_k * mus_k
    # wk[p, c, j] = musk[j, 128c + p]
    wk = np.ascontiguousarray(
        musk.reshape(K, 4, 128).transpose(2, 1, 0)).astype(NPBF16)
    musr = m.astype(NPBF16)
    gram = m @ m.T
    # jittered Cholesky: G = L L^T, |g|^2 = |L^T e|^2
    lmat = np.linalg.cholesky(gram + 1e-9 * np.eye(K)).astype(NPBF16)
    iv2 = np.zeros((128, 2), np.float64)       # [-1/kappa | 0; 0 | 1] stacked
    iv2[0:64, 0] = -1.0 / k
    iv2[64:128, 1] = 1.0
    iv2 = iv2.astype(NPBF16)
    # sel[:, 0:128] selects the r row (rbc); sel[:, 128:256] the rq row (rqbc)
    sel = np.zeros((2, 256), np.float64)
    sel[0, 0:128] = 1.0    # rbc[m, :] = rT2[0, :] = r
    sel[1, 128:256] = 1.0  # rqbc[m, :] = rT2[1, :] = r*(-q)
    sel_r = sel.astype(NPBF16)
    ident = np.eye(128).astype(NPBF16)
    return dict(wk=wk, musr=musr, lmat=lmat, delta=delta, iv2=iv2,
                sel=sel_r, ident=ident)


_NC_CACHE = {}


def kernel(s, alphas, mus, kappas):
    global LAST_RESULT
    s = np.asarray(s, np.float32)
    consts = host_prep(alphas, mus, kappas)

    rows = PAD_ROWS
    if rows not in _NC_CACHE:
        _NC_CACHE[rows] = build_nc(rows)
    nc = _NC_CACHE[rows]

    in_maps = []
    for c in range(N_CORES):
        shard = s[c * ROWS_PER_CORE:(c + 1) * ROWS_PER_CORE]
        pad = rows - shard.shape[0]
        if pad:
            shard = np.concatenate([shard, shard[:pad]], axis=0)
        # per-supertile transpose: DRAM holds s^T blocks [512d, 512r]
        sT = np.ascontiguousarray(
            shard.reshape(N_ST, ST_ROWS, D).transpose(0, 2, 1)
        ).astype(NPBF16).reshape(rows, D)
        in_maps.append({"sT": sT, **consts})

    res = run_bass_kernel_spmd(
        nc, in_maps, list(range(N_CORES)),
        trace=bool(os.environ.get("MIXVMF_TRACE")),
    )
    LAST_RESULT = res
    outs = []
    for c in range(N_CORES):
        oT = np.asarray(res.results[c]["outT"]).reshape(N_ST, D, ST_ROWS)
        o = oT.transpose(0, 2, 1).reshape(rows, D)[:ROWS_PER_CORE]
        outs.append(o.astype(np.float32))
    return np.concatenate(outs, axis=0)
